# revision 1
# baseline (speedup 1.0000x reference)
"""Trainium2 Bass kernel for the soft-logic cellular-automaton nn.Module.

Reference semantics (B=16, M=4096, N=8192, K=6, P=64, L=8, STEP=2):
    tw = sigmoid(toggle_gates)                      # (L, P, N)
    state = zeros(B, N); state[:, ::2] = x
    for l in range(L):
        win[b,n,i] = state[b, (n+i-2) mod N]        # i in 0..5
        w[b,n,p]   = prod_i (bit_i(p) ? win_i : 1-win_i)
        state[b,n] = clip(sum_p w[b,n,p]*tw[l,p,n], 0, 1)
    return state[:, ::2]

Sharding: grid dim N split across 8 cores (1024 owned columns each).
Each core computes a halo-grown region (2 left / 3 right per layer -> 16/24
total) so NO inter-core communication is needed during the 8 layers.

On-core layout ("F-major"): 128 partitions = (b=16) x (chunk c=8); each
partition holds a contiguous 168-column n-window (128 owned + 40 halo) on
the free dim, so the 6 convolution taps are plain AP column offsets.
State+complement live in one paired tile SC[128, 2, W0] (row0 = 1-state,
row1 = state) so a tap selected by a combo bit is a stride-able AP dim.

Per layer the 64-term contraction  sum_p wA[pa]*wB16[pb]*tw[p,n]  (2+4 bit
split: pa = taps 0-1, pb = taps 2-5) runs on the vector engine in 8 big
multi-dim ops using 0-stride broadcast views: 4 pair-tree muls build
wA[4]/wB16[16], two (mul vs tw + segmented tensor_reduce over pb) halves
give g[pa], then one final mul+reduce over pa.  Layer 0 exploits the
stride-2 embedding (odd slots exactly 0/1): only 8 combos per output
parity survive, computed from stride-2 views against a compact 16-combo
toggle tensor.  clip is skipped: tw in (0.5, 0.732) and sum_p w = 1
exactly, so outputs stay inside (0,1).

toggle weights are streamed from DRAM per layer by ONE broadcast DMA
(0-stride b-replication onto all 128 partitions, all 16 SDMA engines),
prefetched TWO layers ahead (bufs=3), and sigmoid-ed in place in two
halves on the scalar engine so consumer muls gate on half the sigmoid.
"""

import os
import sys
from contextlib import ExitStack

import numpy as np

for _p in ("/opt/trn_rl_repo", "/root/.axon_site/_ro/trn_rl_repo"):
    if os.path.isdir(_p) and _p not in sys.path:
        sys.path.insert(0, _p)

import concourse.bass as bass  # noqa: E402
import concourse.tile as tile  # noqa: E402
from concourse import bacc, mybir  # noqa: E402
from concourse.bass_utils import run_bass_kernel_spmd  # noqa: E402

B, M, N, KK, P, L = 16, 4096, 8192, 6, 64, 8
NCORES = 8
NOWN = N // NCORES          # 1024 owned grid columns per core
NCHUNK = 8                  # chunks (partitions per batch row)
CHUNK = NOWN // NCHUNK      # 128 owned columns per partition
GROW_L, GROW_R = 2 * L, 3 * L   # 16, 24
W0 = CHUNK + GROW_L + GROW_R    # 168 column window at layer 0
XW = W0 // 2                    # 84 even columns carrying x
F32 = mybir.dt.float32

# variant knobs: (batched tree/mul via 0-stride views, #pa-groups on gpsimd,
#                 sparse layer-0 using the zero-interleave structure)
DEFAULT_VARIANT = dict(batched=True, gp_pa=0, sparse_l0=True, dve_comp=True,
                       half_l7=True)


def _build_program(reps=1, batched=False, gp_pa=0, sparse_l0=False, probe="",
                   dve_comp=False, sig2=False, addtree=False, half_l7=False,
                   l1split=False):
    nc = bacc.Bacc("TRN2", target_bir_lowering=False, debug=False)
    xs = nc.dram_tensor("xs", [128, XW], F32, kind="ExternalInput").ap()
    tg = nc.dram_tensor("tg", [L, NCHUNK, W0, P], F32, kind="ExternalInput").ap()
    # layer-0 compact toggles: [parity, chunk, e, combo] (16 surviving combos)
    tg0 = nc.dram_tensor("tg0", [NCHUNK, 2, XW, 8], F32, kind="ExternalInput").ap()
    # layer-7 toggles for even output columns only: [chunk, e, p]
    tg7 = nc.dram_tensor("tg7", [NCHUNK, CHUNK // 2, P], F32, kind="ExternalInput").ap()
    # layer-1 toggles pre-split by p-halves: [half, chunk, j, 32]
    tg1 = nc.dram_tensor("tg1", [2, NCHUNK, W0 - 10, 32], F32, kind="ExternalInput").ap()
    out = nc.dram_tensor("out", [128, CHUNK // 2], F32, kind="ExternalOutput").ap()

    mult = mybir.AluOpType.mult
    add = mybir.AluOpType.add
    AX = mybir.AxisListType.X
    AF = mybir.ActivationFunctionType

    with tile.TileContext(nc) as tc, ExitStack() as ctx:
        pool = ctx.enter_context(tc.tile_pool(name="work", bufs=1))
        twpool = ctx.enter_context(tc.tile_pool(name="tw", bufs=3))

        # paired state tiles: row0 = comp (1-state), row1 = state
        SC = [pool.tile([128, 2, W0], F32, name="scA", tag="scA"),
              pool.tile([128, 2, W0], F32, name="scB", tag="scB")]
        tmp4 = pool.tile([128, 4, W0], F32, name="tmp4", tag="tmp4")
        tmp23 = pool.tile([128, 4, W0], F32, name="tmp23", tag="tmp23")
        tmp45 = pool.tile([128, 4, W0], F32, name="tmp45", tag="tmp45")
        wa = pool.tile([128, W0, 8], F32, name="wa", tag="wa")      # (j, pa)
        wb = pool.tile([128, W0, 8], F32, name="wb", tag="wb")      # (j, pb)
        wb16 = pool.tile([128, W0, 16], F32, name="wb16", tag="wb16")  # (j, pb4)
        p64 = pool.tile([128, 2, W0, 16], F32, name="p64", tag="p64")  # (pa, j, pb)
        gf = pool.tile([128, W0, 8], F32, name="gf", tag="gf")      # (j, pa)
        fin = pool.tile([128, W0, 8], F32, name="fin", tag="fin")   # (j, pa)

        # state init: zeros with x at even columns (SWDGE queue: stays off the
        # critical HWDGE queue carrying the first big toggle fetch)
        nc.vector.memset(SC[0][:], 0.0)
        nc.gpsimd.dma_start(out=SC[0][:, 1, 0:W0:2], in_=xs[:, :])

        tw_tiles = {}
        tw_carves = {}

        def pruned(gl):
            return half_l7 and gl % L == L - 1

        def split1(gl):
            return l1split and gl % L == 1

        def fetch_tw(gl):
            t = twpool.tile([128, W0, P], F32, name="twt", tag="tw")
            if probe != "nodma" or gl <= 1:
                if pruned(gl):
                    # host pre-packed even-column toggles, contiguous on chip
                    nc.sync.dma_start(out=t[:, 0:CHUNK // 2, :],
                                      in_=tg7.partition_broadcast(16))
                elif split1(gl):
                    # layer 1: fetch p-halves as two contiguous carves so its
                    # sigmoid halves start after half the DMA (startup head)
                    w1 = W0 - 10
                    tf = t.rearrange("p j q -> p (j q)")
                    cv = []
                    for hh in range(2):
                        c = tf[:, hh * w1 * 32:(hh + 1) * w1 * 32].rearrange(
                            "p (j q) -> p j q", q=32)
                        nc.sync.dma_start(out=c, in_=tg1[hh].partition_broadcast(16))
                        cv.append(c)
                    tw_carves[gl] = cv
                else:
                    ll = gl % L
                    lo, ro = 2 * ll + 2, W0 - 3 * ll - 3
                    # single DMA, b-replication via 0-stride src dim; fetch
                    # only the consumed column window
                    nc.sync.dma_start(
                        out=t[:, lo:ro, :],
                        in_=tg[gl % L][:, lo:ro, :].partition_broadcast(16))
            tw_tiles[gl] = t

        def sigmoid_tw(gl, half):
            if probe == "nosig":
                return
            if split1(gl):
                c = tw_carves[gl][half]
                nc.scalar.activation(c, c, AF.Sigmoid)
                return
            if pruned(gl):
                lo, ro = 0, CHUNK // 2
            else:
                ll = gl % L
                lo, ro = 2 * ll + 2, W0 - 3 * ll - 3
            t = tw_tiles[gl]
            sl = slice(32 * half, 32 * half + 32)
            nc.scalar.activation(t[:, lo:ro, sl], t[:, lo:ro, sl], AF.Sigmoid)

        def needs_tw(gl):
            return gl < L * reps and not (sparse_l0 and gl % L == 0)

        if sparse_l0:
            tw0 = pool.tile([128, 2, XW, 8], F32, name="tw0", tag="tw0")
            nc.gpsimd.dma_start(
                out=tw0[:, :, :, :],
                in_=tg0.partition_broadcast(16))
            nc.scalar.activation(tw0[:, :, :, :], tw0[:, :, :, :], AF.Sigmoid)
        else:
            fetch_tw(0)
            sigmoid_tw(0, 0)
            sigmoid_tw(0, 1)
        if needs_tw(1):
            fetch_tw(1)
            if sig2:
                sigmoid_tw(1, 0)
                sigmoid_tw(1, 1)

        for gl in range(L * reps):
            l = gl % L
            lin, rin = 2 * l, W0 - 3 * l
            lo, ro = lin + 2, rin - 3
            wo = ro - lo
            sin, sout = SC[gl % 2], SC[(gl + 1) % 2]

            # prefetch toggle gates TWO layers ahead (bufs=3) so next layer's
            # sigmoid never waits on its DMA
            if needs_tw(gl + 2):
                fetch_tw(gl + 2)

            # comp = 1 - state on the input window. On DVE (tensor_scalar,
            # single-src 2x path) the fin-reduce -> comp -> tree chain stays
            # on one engine: no cross-engine semaphore bubble per layer, and
            # ACT's FIFO holds only sigmoids.
            if dve_comp:
                nc.vector.tensor_scalar(sin[:, 0, lin:rin], sin[:, 1, lin:rin],
                                        -1.0, 1.0, mult, add)
            else:
                nc.scalar.activation(sin[:, 0, lin:rin], sin[:, 1, lin:rin],
                                     AF.Identity, bias=1.0, scale=-1.0)

            # sigmoid queues on ACT in two halves so consumer big-muls gate
            # on half the work; with sig2 it runs a full extra layer early
            sgl = gl + 2 if sig2 else gl + 1
            if needs_tw(sgl):
                sigmoid_tw(sgl, 0)
                sigmoid_tw(sgl, 1)

            if sparse_l0 and l == 0:
                # Layer 0: odd grid slots are exactly 0 (state) / 1 (comp), so
                # only 8 of 64 combos survive per output parity; taps collapse
                # to stride-2 views of the x-carrying even slots.
                # even outputs j=2e, e in [1,82]: taps at even slots e-1,e,e+1
                VE = [sin[:, :, 2 * d: 2 * d + 164: 2] for d in (0, 1, 2)]
                t4e = tmp4.rearrange("p (a b) j -> p a b j", a=2)[:, :, :, 0:82]
                nc.vector.tensor_tensor(
                    t4e,
                    VE[0].unsqueeze(2).broadcast_to((128, 2, 2, 82)),
                    VE[1].unsqueeze(1).broadcast_to((128, 2, 2, 82)), mult)
                wav = wa[:, 0:82, 0:8].rearrange("p j (q b) -> p q b j", q=4)
                nc.vector.tensor_tensor(
                    wav,
                    tmp4[:, :, 0:82].unsqueeze(2).broadcast_to((128, 4, 2, 82)),
                    VE[2].unsqueeze(1).broadcast_to((128, 4, 2, 82)),
                    mult)
                nc.vector.tensor_tensor(p64[:, 0, 0:82, 0:8], wa[:, 0:82, :],
                                        tw0[:, 0, 1:83, :], mult)
                nc.vector.tensor_reduce(sout[:, 1, 2:165:2], p64[:, 0, 0:82, 0:8],
                                        axis=AX, op=add)
                # odd outputs j=2e+1, e in [1,81]: taps at even slots e,e+1,e+2
                VO = [sin[:, :, 2 * d + 2: 2 * d + 164: 2] for d in (0, 1, 2)]
                t4o = tmp4.rearrange("p (a b) j -> p a b j", a=2)[:, :, :, 0:81]
                nc.vector.tensor_tensor(
                    t4o,
                    VO[0][:, :, 0:81].unsqueeze(2).broadcast_to((128, 2, 2, 81)),
                    VO[1][:, :, 0:81].unsqueeze(1).broadcast_to((128, 2, 2, 81)), mult)
                wbv = wb[:, 0:81, 0:8].rearrange("p j (q b) -> p q b j", q=4)
                nc.vector.tensor_tensor(
                    wbv,
                    tmp4[:, :, 0:81].unsqueeze(2).broadcast_to((128, 4, 2, 81)),
                    VO[2][:, :, 0:81].unsqueeze(1).broadcast_to((128, 4, 2, 81)),
                    mult)
                nc.vector.tensor_tensor(p64[:, 1, 0:81, 0:8], wb[:, 0:81, :],
                                        tw0[:, 1, 1:82, :], mult)
                nc.vector.tensor_reduce(sout[:, 1, 3:164:2], p64[:, 1, 0:81, 0:8],
                                        axis=AX, op=add)
                continue

            twl = tw_tiles[gl]

            def V(i, bit):
                # [128, wo] view of tap i (bit=1: state, 0: comp)
                return sin[:, bit, lin + i: lin + i + wo]

            # last layer: only even grid columns are ever read out, so
            # compute just those (all views become stride-2; volume halves)
            js = 2 if (half_l7 and l == L - 1) else 1
            wos = wo // js

            def VP(i):
                # [128, 2, wos] view of tap i, dim1 selects comp/state
                return sin[:, :, lin + i: lin + i + wo: js]

            if batched:
                # --- 2+4 bit split: wA = taps 0,1 (4 combos, = tmp4),
                #     wB16 = taps 2..5 (16 combos) built from two pair trees ---
                t4v = tmp4.rearrange("p (a b) j -> p a b j", a=2)[:, :, :, 0:wos]
                nc.vector.tensor_tensor(
                    t4v,
                    VP(0).unsqueeze(2).broadcast_to((128, 2, 2, wos)),
                    VP(1).unsqueeze(1).broadcast_to((128, 2, 2, wos)), mult)
                t23v = tmp23.rearrange("p (a b) j -> p a b j", a=2)[:, :, :, 0:wos]
                nc.vector.tensor_tensor(
                    t23v,
                    VP(2).unsqueeze(2).broadcast_to((128, 2, 2, wos)),
                    VP(3).unsqueeze(1).broadcast_to((128, 2, 2, wos)), mult)
                t45v = tmp45.rearrange("p (a b) j -> p a b j", a=2)[:, :, :, 0:wos]
                nc.vector.tensor_tensor(
                    t45v,
                    VP(4).unsqueeze(2).broadcast_to((128, 2, 2, wos)),
                    VP(5).unsqueeze(1).broadcast_to((128, 2, 2, wos)), mult)
                wb16v = wb16[:, 0:wos, :].rearrange("p j (q b) -> p q b j", q=4)
                nc.vector.tensor_tensor(
                    wb16v,
                    tmp23[:, :, 0:wos].unsqueeze(2).broadcast_to((128, 4, 4, wos)),
                    tmp45[:, :, 0:wos].unsqueeze(1).broadcast_to((128, 4, 4, wos)),
                    mult)

                # --- products vs tw + segmented reduce, in two halves gated on
                #     the two sigmoid halves ---
                for h in range(2):
                    nc.vector.tensor_tensor(
                        p64[:, :, 0:wos, :],
                        wb16[:, 0:wos, :].unsqueeze(1).broadcast_to(
                            (128, 2, wos, 16)),
                        (tw_carves[gl][h] if split1(gl) else
                         (twl[:, 0:wos, :] if js == 2 else twl[:, lo:ro, :])
                         [:, :, 32 * h:32 * h + 32]).rearrange(
                            "p j (a b) -> p a j b", a=2), mult)
                    gv = gf[:, 0:wos, 2 * h:2 * h + 2].rearrange("p j a -> p a j")
                    if addtree:
                        # pairwise in-place TT adds instead of tensor_reduce
                        for wdt in (8, 4, 2):
                            nc.vector.tensor_tensor(
                                p64[:, :, 0:wos, 0:wdt], p64[:, :, 0:wos, 0:wdt],
                                p64[:, :, 0:wos, wdt:2 * wdt], add)
                        nc.vector.tensor_tensor(
                            gv, p64[:, :, 0:wos, 0], p64[:, :, 0:wos, 1], add)
                    else:
                        nc.vector.tensor_reduce(
                            gv, p64[:, :, 0:wos, :], axis=AX, op=add)

                # --- out = sum_{pa in 4} wA[pa] * g[pa] ---
                nc.vector.tensor_tensor(
                    fin[:, 0:wos, 0:4].rearrange("p j a -> p a j"),
                    tmp4[:, :, 0:wos],
                    gf[:, 0:wos, 0:4].rearrange("p j a -> p a j"), mult)
                nc.vector.tensor_reduce(sout[:, 1, lo:ro:js], fin[:, 0:wos, 0:4],
                                        axis=AX, op=add)
                continue
            else:
                for q in range(4):
                    nc.vector.tensor_tensor(tmp4[:, q, 0:wo], V(0, q >> 1), V(1, q & 1), mult)
                for pa in range(8):
                    nc.vector.tensor_tensor(wa[:, 0:wo, pa], tmp4[:, pa >> 1, 0:wo], V(2, pa & 1), mult)
                for q in range(4):
                    nc.vector.tensor_tensor(tmp4[:, q, 0:wo], V(3, q >> 1), V(4, q & 1), mult)
                for pb in range(8):
                    nc.vector.tensor_tensor(wb[:, 0:wo, pb], tmp4[:, pb >> 1, 0:wo], V(5, pb & 1), mult)

            # --- unbatched fallback: per-pa products vs tw, gpsimd takes the
            #     LAST gp_pa groups (p64 viewed as 8 groups of 8) ---
            def pv(pa):
                return p64[:, (pa >> 1) & 1, 0:wo, 8 * (pa & 1):8 * (pa & 1) + 8]

            for pa in range(8):
                eng = nc.gpsimd if pa >= 8 - gp_pa else nc.vector
                eng.tensor_tensor(
                    pv(pa), wb[:, 0:wo, :],
                    twl[:, lo:ro, pa * 8:(pa + 1) * 8], mult)
                nc.vector.tensor_reduce(
                    gf[:, 0:wo, pa], pv(pa), axis=AX, op=add)

            # --- out = sum_pa wA[pa] * g[pa] ---
            nc.vector.tensor_tensor(fin[:, 0:wo, :], wa[:, 0:wo, :], gf[:, 0:wo, :], mult)
            nc.vector.tensor_reduce(sout[:, 1, lo:ro], fin[:, 0:wo, 0:8], axis=AX, op=add)

        # owned even columns -> output
        nc.sync.dma_start(out=out, in_=SC[(L * reps) % 2][:, 1, GROW_L:GROW_L + CHUNK:2])

    nc.compile()
    return nc


_prog_cache = {}


def _get_program(reps=1, **variant):
    v = dict(DEFAULT_VARIANT)
    v.update(variant)
    key = (reps, tuple(sorted(v.items())))
    if key not in _prog_cache:
        _prog_cache[key] = _build_program(reps, **v)
    return _prog_cache[key]


def _shard_inputs(x, toggle_gates):
    x = np.ascontiguousarray(x, dtype=np.float32)
    tg = np.ascontiguousarray(toggle_gates, dtype=np.float32)
    in_maps = []
    c = np.arange(NCHUNK)
    j = np.arange(W0)
    # layer-0 surviving combos (even outputs: bits 1,3,5 = 0; odd: bits 0,2,4 = 0)
    p_even = np.array([32 * (q >> 2) + 8 * ((q >> 1) & 1) + 2 * (q & 1)
                       for q in range(8)])
    p_odd = np.array([16 * (q >> 2) + 4 * ((q >> 1) & 1) + (q & 1)
                      for q in range(8)])
    for k in range(NCORES):
        n0 = k * NOWN
        nglob = (n0 + CHUNK * c[:, None] - GROW_L + j[None, :]) % N  # [8, 168]
        m_idx = nglob[:, 0::2] // 2                                   # [8, 84]
        xs = x[:, m_idx].reshape(B * NCHUNK, XW)                      # [128, 84]
        tgk = tg[:, :, nglob]                                         # [L, P, 8, 168]
        tg0 = np.stack([tgk[0, p_even][:, :, 0::2],                   # [8q, 8c, 84]
                        tgk[0, p_odd][:, :, 1::2]])                   # [2, 8q, 8c, 84]
        tg0 = np.ascontiguousarray(tg0.transpose(2, 0, 3, 1))         # [8c, 2, 84, 8q]
        tg7 = np.ascontiguousarray(
            tgk[L - 1][:, :, GROW_L:GROW_L + CHUNK:2].transpose(1, 2, 0))  # [8c, 64e, P]
        tg1 = np.ascontiguousarray(
            tgk[1][:, :, 4:W0 - 6].reshape(2, 32, NCHUNK, W0 - 10)
            .transpose(0, 2, 3, 1))                                    # [2, 8c, 158, 32]
        tgk = np.ascontiguousarray(tgk.transpose(0, 2, 3, 1))         # [L, 8, 168, P]
        in_maps.append({"xs": np.ascontiguousarray(xs), "tg": tgk, "tg0": tg0,
                        "tg7": tg7, "tg1": tg1})
    return in_maps


def _run(x, toggle_gates, trace=False, reps=1, **kw):
    nc = _get_program(reps, **kw)
    in_maps = _shard_inputs(x, toggle_gates)
    res = run_bass_kernel_spmd(nc, in_maps, list(range(NCORES)), trace=trace)
    y = np.empty((B, M), dtype=np.float32)
    for k in range(NCORES):
        o = np.asarray(res.results[k]["out"]).reshape(B, NCHUNK * CHUNK // 2)
        y[:, k * (NOWN // 2):(k + 1) * (NOWN // 2)] = o
    return y, res


def kernel(x, toggle_gates):
    # Retry-then-fallback ladder: both variants are hardware-verified; a
    # transient device error (e.g. NRT_EXEC_UNIT_UNRECOVERABLE was observed
    # once during development) should not zero the run.  The fastest variant
    # is tried twice before stepping down.
    ladder = [
        dict(DEFAULT_VARIANT),
        dict(DEFAULT_VARIANT, half_l7=False, l1split=False),
    ]
    last_err = None
    for v in ladder:
        for _attempt in range(2):
            try:
                y, _ = _run(x, toggle_gates, **v)
                return y
            except Exception as e:  # noqa: BLE001 - deliberate catch-all retry
                last_err = e
    raise last_err



# revision 3
# speedup vs baseline: 1.7445x; 1.7445x over previous
"""Trainium2 Bass kernel for the soft-logic cellular-automaton nn.Module.

Reference semantics (B=16, M=4096, N=8192, K=6, P=64, L=8, STEP=2):
    tw = sigmoid(toggle_gates)                      # (L, P, N)
    state = zeros(B, N); state[:, ::2] = x
    for l in range(L):
        win[b,n,i] = state[b, (n+i-2) mod N]        # i in 0..5
        w[b,n,p]   = prod_i (bit_i(p) ? win_i : 1-win_i)
        state[b,n] = clip(sum_p w[b,n,p]*tw[l,p,n], 0, 1)
    return state[:, ::2]

Sharding: grid dim N split across 8 cores (1024 owned columns each).
Each core computes a halo-grown region (2 left / 3 right per layer -> 16/24
total) so NO inter-core communication is needed during the 8 layers.

On-core layout ("F-major"): 128 partitions = (b=16) x (chunk c=8); each
partition holds a contiguous 168-column n-window (128 owned + 40 halo) on
the free dim. State+complement live in one paired fp16 tile SC[128, 2, W0]
(row0 = 1-state, row1 = state).

The whole datapath is fp16: on TRN2's DVE, tensor_tensor with all-2-byte
packed (stride-1 innermost) operands runs in 2x mode and tensor_scalar in
4x mode, while tensor_reduce never gets a fast mode.  So the 64-term
contraction  sum_p wA[pa]*wB16[pb]*tw[p,n]  (2+4 bit split) is computed as
fp16 broadcast-view products into a combo-MAJOR p64[128, 2, 16pb, n] tile
followed by a pairwise in-place add-tree over pb (j stays innermost at
every level -> every add runs 2x), then a 4-term fp16 combine.  Per-op
fp16 rounding was simulated end-to-end: max rel err ~1.5e-3, well inside
the 2e-2 gate (fp32 internal ALU accumulate, rounding only on write).

Layer 0 exploits the stride-2 embedding (odd slots exactly 0/1): only 8
combos per output parity survive, computed from a COMPACT x tile (stride-1,
2x) against compact 16-combo toggles.  Layer 7 computes only the even
(read-out) columns from compact stride-1 parity copies of the state and
writes the final fp32 output tile directly.  clip is skipped: tw in
(0.5, 0.732) and sum_p w = 1 exactly, so outputs stay inside (0,1).

toggle weights are packed fp16 combo-major on the host, streamed per layer
by ONE broadcast DMA (0-stride b-replication onto all 128 partitions),
prefetched TWO layers ahead (bufs=3), and sigmoid-ed in place in two
32-combo halves on the scalar engine so consumer muls gate on half the
sigmoid.
"""

import os
import sys
from contextlib import ExitStack

import numpy as np

for _p in ("/opt/trn_rl_repo", "/root/.axon_site/_ro/trn_rl_repo"):
    if os.path.isdir(_p) and _p not in sys.path:
        sys.path.insert(0, _p)

import concourse.bass as bass  # noqa: E402
import concourse.tile as tile  # noqa: E402
from concourse import bacc, mybir  # noqa: E402
from concourse.bass_utils import run_bass_kernel_spmd  # noqa: E402

B, M, N, KK, P, L = 16, 4096, 8192, 6, 64, 8
NCORES = 8
NOWN = N // NCORES          # 1024 owned grid columns per core
NCHUNK = 8                  # chunks (partitions per batch row)
CHUNK = NOWN // NCHUNK      # 128 owned columns per partition
GROW_L, GROW_R = 2 * L, 3 * L   # 16, 24
W0 = CHUNK + GROW_L + GROW_R    # 168 column window at layer 0
XW = W0 // 2                    # 84 even columns carrying x
F16 = mybir.dt.float16
F32 = mybir.dt.float32

DEFAULT_VARIANT = dict(sparse_l0=True, half_l7=True)


def _build_program(reps=1, sparse_l0=True, half_l7=True, probe=""):
    nc = bacc.Bacc("TRN2", target_bir_lowering=False, debug=False)
    xs = nc.dram_tensor("xs", [128, XW], F16, kind="ExternalInput").ap()
    # combo-major toggles: [layer, chunk, combo, col]
    tg = nc.dram_tensor("tg", [L, NCHUNK, P, W0], F16, kind="ExternalInput").ap()
    # layer-0 compact toggles: [chunk, parity, combo(8), e]
    tg0 = nc.dram_tensor("tg0", [NCHUNK, 2, 8, XW], F16, kind="ExternalInput").ap()
    # layer-7 toggles for even output columns only: [chunk, combo, e]
    tg7 = nc.dram_tensor("tg7", [NCHUNK, P, CHUNK // 2], F16, kind="ExternalInput").ap()
    out = nc.dram_tensor("out", [128, CHUNK // 2], F32, kind="ExternalOutput").ap()

    mult = mybir.AluOpType.mult
    add = mybir.AluOpType.add
    AF = mybir.ActivationFunctionType

    with tile.TileContext(nc) as tc, ExitStack() as ctx:
        pool = ctx.enter_context(tc.tile_pool(name="work", bufs=1))
        twpool = ctx.enter_context(tc.tile_pool(name="tw", bufs=3))

        # paired state tiles: row0 = comp (1-state), row1 = state
        SC = [pool.tile([128, 2, W0], F16, name="scA", tag="scA"),
              pool.tile([128, 2, W0], F16, name="scB", tag="scB")]
        t4 = pool.tile([128, 2, 2, W0], F16, name="t4", tag="t4")
        t23 = pool.tile([128, 2, 2, W0], F16, name="t23", tag="t23")
        t45 = pool.tile([128, 2, 2, W0], F16, name="t45", tag="t45")
        wb16 = pool.tile([128, 4, 4, W0], F16, name="wb16", tag="wb16")
        p64 = pool.tile([128, 2, 16, W0], F16, name="p64", tag="p64")
        gf = pool.tile([128, 4, W0], F16, name="gf", tag="gf")
        fin = pool.tile([128, 4, W0], F16, name="fin", tag="fin")
        # compact stride-1 parity copies of state for the half layer 7
        cpar = pool.tile([128, 2, 2, XW], F16, name="cpar", tag="cpar")
        o32 = pool.tile([128, CHUNK // 2], F32, name="o32", tag="o32")

        # state init: zeros with x at even columns (SWDGE queue: stays off the
        # critical HWDGE queue carrying the first big toggle fetch)
        nc.vector.memset(SC[0][:], 0.0)
        nc.gpsimd.dma_start(out=SC[0][:, 1, 0:W0:2], in_=xs[:, :])

        tw_tiles = {}

        def pruned(gl):
            return half_l7 and gl % L == L - 1

        def fetch_tw(gl):
            t = twpool.tile([128, P, W0], F16, name="twt", tag="tw")
            if probe != "nodma" or gl <= 1:
                if pruned(gl):
                    # host pre-packed even-column toggles, contiguous on chip
                    nc.sync.dma_start(out=t[:, :, 0:CHUNK // 2],
                                      in_=tg7.partition_broadcast(16))
                else:
                    ll = gl % L
                    lo, ro = 2 * ll + 2, W0 - 3 * ll - 3
                    # single DMA, b-replication via 0-stride src dim; fetch
                    # only the consumed column window
                    nc.sync.dma_start(
                        out=t[:, :, lo:ro],
                        in_=tg[gl % L][:, :, lo:ro].partition_broadcast(16))
            tw_tiles[gl] = t

        def sigmoid_tw(gl, half):
            if probe == "nosig":
                return
            if pruned(gl):
                lo, ro = 0, CHUNK // 2
            else:
                ll = gl % L
                lo, ro = 2 * ll + 2, W0 - 3 * ll - 3
            t = tw_tiles[gl]
            nc.scalar.activation(t[:, 32 * half:32 * half + 32, lo:ro],
                                 t[:, 32 * half:32 * half + 32, lo:ro],
                                 AF.Sigmoid)

        def needs_tw(gl):
            return gl < L * reps and not (sparse_l0 and gl % L == 0)

        if sparse_l0:
            tw0 = pool.tile([128, 2, 8, XW], F16, name="tw0", tag="tw0")
            nc.gpsimd.dma_start(out=tw0[:], in_=tg0.partition_broadcast(16))
            nc.scalar.activation(tw0[:], tw0[:], AF.Sigmoid)
        else:
            fetch_tw(0)
            sigmoid_tw(0, 0)
            sigmoid_tw(0, 1)
        if needs_tw(1):
            fetch_tw(1)

        for gl in range(L * reps):
            l = gl % L
            lin, rin = 2 * l, W0 - 3 * l
            lo, ro = lin + 2, rin - 3
            wos = ro - lo
            sin, sout = SC[gl % 2], SC[(gl + 1) % 2]

            # prefetch toggle gates TWO layers ahead (bufs=3) so next layer's
            # sigmoid never waits on its DMA
            if needs_tw(gl + 2):
                fetch_tw(gl + 2)

            # comp = 1 - state on the input window (fp16 tensor_scalar: 4x)
            nc.vector.tensor_scalar(sin[:, 0, lin:rin], sin[:, 1, lin:rin],
                                    -1.0, 1.0, mult, add)

            # sigmoid queues on ACT in two 32-combo halves so consumer
            # big-muls gate on half the work
            if needs_tw(gl + 1):
                sigmoid_tw(gl + 1, 0)
                sigmoid_tw(gl + 1, 1)

            if sparse_l0 and l == 0:
                # Layer 0: odd grid slots are exactly 0 (state) / 1 (comp), so
                # only 8 of 64 combos survive per output parity; taps collapse
                # to stride-1 views of a COMPACT x tile cpar[:, 0] with
                # dim 0=comp, 1=state of the 84 x-carrying even slots.
                nc.vector.tensor_scalar(cpar[:, 0, 1, :], sin[:, 1, 0:W0:2],
                                        1.0, 0.0, mult, add)
                nc.vector.tensor_scalar(cpar[:, 0, 0, :], cpar[:, 0, 1, :],
                                        -1.0, 1.0, mult, add)
                X = cpar[:, 0]  # [128, 2, XW]: dim1 0=comp, 1=state

                for par, ne in ((0, 82), (1, 81)):
                    # even outputs j=2e, e in [1,82]: taps X[e-1], X[e], X[e+1]
                    # odd outputs j=2e+1, e in [1,81]: taps X[e], X[e+1], X[e+2]
                    V = [X[:, :, d + par: d + par + ne] for d in (0, 1, 2)]
                    tp = t4[:, :, :, 0:ne]
                    nc.vector.tensor_tensor(
                        tp,
                        V[0].unsqueeze(2).broadcast_to((128, 2, 2, ne)),
                        V[1].unsqueeze(1).broadcast_to((128, 2, 2, ne)), mult)
                    w8 = wb16.rearrange("p a b j -> p (a b) j") \
                        .rearrange("p (q c) j -> p q c j", c=2)[:, 0:4, :, 0:ne]
                    nc.vector.tensor_tensor(
                        w8,
                        t4.rearrange("p a b j -> p (a b) j")[:, :, 0:ne]
                        .unsqueeze(2).broadcast_to((128, 4, 2, ne)),
                        V[2].unsqueeze(1).broadcast_to((128, 4, 2, ne)), mult)
                    tw0v = tw0[:, par].rearrange("p (q c) j -> p q c j", c=2)
                    nc.vector.tensor_tensor(w8, w8,
                                            tw0v[:, :, :, 1:1 + ne], mult)
                    nc.vector.tensor_tensor(w8[:, 0:2], w8[:, 0:2],
                                            w8[:, 2:4], add)
                    nc.vector.tensor_tensor(w8[:, 0, :, :], w8[:, 0, :, :],
                                            w8[:, 1, :, :], add)
                    nc.vector.tensor_tensor(
                        sout[:, 1, 2 + par:2 + par + 2 * ne:2],
                        w8[:, 0, 0, :], w8[:, 0, 1, :], add)
                continue

            twl = tw_tiles[gl]
            half7 = half_l7 and l == L - 1

            if half7:
                # compact stride-1 parity copies: even-col taps 0,2,4 and
                # odd-col taps 1,3,5 (output cols j=lo..ro step 2, wos evens)
                wos = wos // 2
                nce = wos + 3
                nc.vector.tensor_scalar(
                    cpar[:, 0, :, 0:nce],
                    sin[:, :, lin: lin + 2 * nce: 2], 1.0, 0.0, mult, add)
                nc.vector.tensor_scalar(
                    cpar[:, 1, :, 0:nce],
                    sin[:, :, lin + 1: lin + 1 + 2 * nce: 2], 1.0, 0.0,
                    mult, add)

                def VP(i):
                    return cpar[:, i % 2, :, i // 2: i // 2 + wos]
            else:
                def VP(i):
                    return sin[:, :, lin + i: lin + i + wos]

            # --- 2+4 bit split: wA = taps 0,1 (4 combos, = t4),
            #     wB16 = taps 2..5 (16 combos) from two pair trees, combo-major
            nc.vector.tensor_tensor(
                t4[:, :, :, 0:wos],
                VP(0).unsqueeze(2).broadcast_to((128, 2, 2, wos)),
                VP(1).unsqueeze(1).broadcast_to((128, 2, 2, wos)), mult)
            nc.vector.tensor_tensor(
                t23[:, :, :, 0:wos],
                VP(2).unsqueeze(2).broadcast_to((128, 2, 2, wos)),
                VP(3).unsqueeze(1).broadcast_to((128, 2, 2, wos)), mult)
            nc.vector.tensor_tensor(
                t45[:, :, :, 0:wos],
                VP(4).unsqueeze(2).broadcast_to((128, 2, 2, wos)),
                VP(5).unsqueeze(1).broadcast_to((128, 2, 2, wos)), mult)
            nc.vector.tensor_tensor(
                wb16[:, :, :, 0:wos],
                t23.rearrange("p a b j -> p (a b) j")[:, :, 0:wos]
                .unsqueeze(2).broadcast_to((128, 4, 4, wos)),
                t45.rearrange("p a b j -> p (a b) j")[:, :, 0:wos]
                .unsqueeze(1).broadcast_to((128, 4, 4, wos)), mult)

            # --- per tw-half: products then pairwise pb add-tree (all views
            #     keep j innermost stride-1 -> every op runs the fp16 2x path)
            for h in range(2):
                tv = (twl[:, 32 * h:32 * h + 32, 0:wos] if half7 else
                      twl[:, 32 * h:32 * h + 32, lo:ro])
                nc.vector.tensor_tensor(
                    p64[:, :, :, 0:wos],
                    wb16.rearrange("p a b j -> p (a b) j")[:, :, 0:wos]
                    .unsqueeze(1).broadcast_to((128, 2, 16, wos)),
                    tv.rearrange("p (a b) j -> p a b j", a=2), mult)
                for w_ in (8, 4, 2):
                    nc.vector.tensor_tensor(
                        p64[:, :, 0:w_, 0:wos], p64[:, :, 0:w_, 0:wos],
                        p64[:, :, w_:2 * w_, 0:wos], add)
                nc.vector.tensor_tensor(
                    gf[:, 2 * h:2 * h + 2, 0:wos],
                    p64[:, :, 0, 0:wos], p64[:, :, 1, 0:wos], add)

            # --- out = sum_{pa in 4} wA[pa] * g[pa] ---
            nc.vector.tensor_tensor(
                fin[:, :, 0:wos],
                t4.rearrange("p a b j -> p (a b) j")[:, :, 0:wos],
                gf[:, :, 0:wos], mult)
            nc.vector.tensor_tensor(fin[:, 0:2, 0:wos], fin[:, 0:2, 0:wos],
                                    fin[:, 2:4, 0:wos], add)
            if half7:
                # layer 7 computes exactly the owned even columns: write the
                # fp32 output tile directly
                nc.vector.tensor_tensor(o32[:, :], fin[:, 0, 0:wos],
                                        fin[:, 1, 0:wos], add)
            else:
                nc.vector.tensor_tensor(sout[:, 1, lo:ro], fin[:, 0, 0:wos],
                                        fin[:, 1, 0:wos], add)

        if not (half_l7):
            # owned even columns -> fp32 output
            nc.vector.tensor_scalar(
                o32[:, :], SC[(L * reps) % 2][:, 1, GROW_L:GROW_L + CHUNK:2],
                1.0, 0.0, mult, add)
        nc.sync.dma_start(out=out, in_=o32[:, :])

    nc.compile()
    return nc


_prog_cache = {}


def _get_program(reps=1, **variant):
    v = dict(DEFAULT_VARIANT)
    v.update(variant)
    key = (reps, tuple(sorted(v.items())))
    if key not in _prog_cache:
        _prog_cache[key] = _build_program(reps, **v)
    return _prog_cache[key]


def _shard_inputs(x, toggle_gates):
    x = np.ascontiguousarray(x, dtype=np.float32)
    tg = np.ascontiguousarray(toggle_gates, dtype=np.float32)
    in_maps = []
    c = np.arange(NCHUNK)
    j = np.arange(W0)
    # layer-0 surviving combos (even outputs: bits 1,3,5 = 0; odd: bits 0,2,4 = 0)
    p_even = np.array([32 * (q >> 2) + 8 * ((q >> 1) & 1) + 2 * (q & 1)
                       for q in range(8)])
    p_odd = np.array([16 * (q >> 2) + 4 * ((q >> 1) & 1) + (q & 1)
                      for q in range(8)])
    for k in range(NCORES):
        n0 = k * NOWN
        nglob = (n0 + CHUNK * c[:, None] - GROW_L + j[None, :]) % N  # [8, 168]
        m_idx = nglob[:, 0::2] // 2                                   # [8, 84]
        xs = x[:, m_idx].reshape(B * NCHUNK, XW)                      # [128, 84]
        tgk = tg[:, :, nglob]                                         # [L, P, 8, 168]
        tg0 = np.stack([tgk[0, p_even][:, :, 0::2],                   # [8q, 8c, 84]
                        tgk[0, p_odd][:, :, 1::2]])                   # [2, 8q, 8c, 84]
        tg0 = np.ascontiguousarray(tg0.transpose(2, 0, 1, 3))         # [8c, 2, 8q, 84]
        tg7 = np.ascontiguousarray(
            tgk[L - 1][:, :, GROW_L:GROW_L + CHUNK:2].transpose(1, 0, 2))  # [8c, P, 64e]
        tgk = np.ascontiguousarray(tgk.transpose(0, 2, 1, 3))         # [L, 8, P, 168]
        in_maps.append({"xs": np.ascontiguousarray(xs).astype(np.float16),
                        "tg": tgk.astype(np.float16),
                        "tg0": tg0.astype(np.float16),
                        "tg7": tg7.astype(np.float16)})
    return in_maps


def _run(x, toggle_gates, trace=False, reps=1, **kw):
    nc = _get_program(reps, **kw)
    in_maps = _shard_inputs(x, toggle_gates)
    res = run_bass_kernel_spmd(nc, in_maps, list(range(NCORES)), trace=trace)
    y = np.empty((B, M), dtype=np.float32)
    for k in range(NCORES):
        o = np.asarray(res.results[k]["out"]).reshape(B, NCHUNK * CHUNK // 2)
        y[:, k * (NOWN // 2):(k + 1) * (NOWN // 2)] = o
    return y, res


def kernel(x, toggle_gates):
    # Retry-then-fallback ladder: a transient device error (e.g.
    # NRT_EXEC_UNIT_UNRECOVERABLE was observed once during development)
    # should not zero the run.  The fastest variant is tried twice before
    # stepping down to the plainer one.
    ladder = [
        dict(DEFAULT_VARIANT),
        dict(DEFAULT_VARIANT, sparse_l0=False, half_l7=False),
    ]
    last_err = None
    for v in ladder:
        for _attempt in range(2):
            try:
                y, _ = _run(x, toggle_gates, **v)
                return y
            except Exception as e:  # noqa: BLE001 - deliberate catch-all retry
                last_err = e
    raise last_err


# revision 5
# speedup vs baseline: 2.0128x; 1.1538x over previous
"""Trainium2 Bass kernel for the soft-logic cellular-automaton nn.Module.

Reference semantics (B=16, M=4096, N=8192, K=6, P=64, L=8, STEP=2):
    tw = sigmoid(toggle_gates)                      # (L, P, N)
    state = zeros(B, N); state[:, ::2] = x
    for l in range(L):
        win[b,n,i] = state[b, (n+i-2) mod N]        # i in 0..5
        w[b,n,p]   = prod_i (bit_i(p) ? win_i : 1-win_i)
        state[b,n] = clip(sum_p w[b,n,p]*tw[l,p,n], 0, 1)
    return state[:, ::2]

Sharding: grid dim N split across 8 cores (1024 owned columns each).
Each core computes a halo-grown region (2 left / 3 right per layer -> 16/24
total) so NO inter-core communication is needed during the 8 layers.

On-core layout ("F-major"): 128 partitions = (b=16) x (chunk c=8); each
partition holds a contiguous 168-column n-window (128 owned + 40 halo) on
the free dim. State+complement live in one paired fp16 tile SC[128, 2, W0]
(row0 = 1-state, row1 = state).

The whole datapath is fp16: on TRN2's DVE, tensor_tensor with all-2-byte
packed (stride-1 innermost) operands runs in 2x mode and tensor_scalar in
4x mode, while tensor_reduce never gets a fast mode.  So the 64-term
contraction  sum_p wA[pa]*wB16[pb]*tw[p,n]  (2+4 bit split) is computed as
fp16 broadcast-view products into a combo-MAJOR p64[128, 2, 16pb, n] tile
followed by a pairwise in-place add-tree over pb (j stays innermost at
every level -> every add runs 2x), then a 4-term fp16 combine.  Per-op
fp16 rounding was simulated end-to-end: max rel err ~1.6e-3, well inside
the 2e-2 gate (fp32 internal ALU accumulate, rounding only on write).

toggle gates are affine-quantized to uint8 on the host (q = (g-lo)/(hi-lo)
*255) and streamed per layer as ONE-contiguous-run-per-partition broadcast
DMAs in two 32-combo halves (contiguous runs >= 512B avoid the DMA's 2x
small-descriptor penalty; uint8 halves the bus bytes again -> ~4us/layer
vs ~13us for the naive strided fp32 fetch).  The scalar engine dequantizes
+ applies sigmoid in one pass per half (out fp16) with per-partition
scale/bias scalars shipped as a tiny input tensor, so the compiled program
stays input-independent.  Fetches are prefetched TWO layers ahead.

Layer 0 exploits the stride-2 embedding (odd slots exactly 0/1): only 8
combos per output parity survive, computed from a COMPACT x tile (stride-1,
2x) against compact 16-combo toggles.  Layer 7 computes only the even
(read-out) columns from compact stride-1 parity copies of the state and
writes the final fp32 output tile directly.  clip is skipped: tw in
(0.5, 0.732) and sum_p w = 1 exactly, so outputs stay inside (0,1).
"""

import os
import sys
from contextlib import ExitStack

import numpy as np

for _p in ("/opt/trn_rl_repo", "/root/.axon_site/_ro/trn_rl_repo"):
    if os.path.isdir(_p) and _p not in sys.path:
        sys.path.insert(0, _p)

import concourse.bass as bass  # noqa: E402
import concourse.tile as tile  # noqa: E402
from concourse import bacc, mybir  # noqa: E402
from concourse.bass_utils import run_bass_kernel_spmd  # noqa: E402

B, M, N, KK, P, L = 16, 4096, 8192, 6, 64, 8
NCORES = 8
NOWN = N // NCORES          # 1024 owned grid columns per core
NCHUNK = 8                  # chunks (partitions per batch row)
CHUNK = NOWN // NCHUNK      # 128 owned columns per partition
GROW_L, GROW_R = 2 * L, 3 * L   # 16, 24
W0 = CHUNK + GROW_L + GROW_R    # 168 column window at layer 0
XW = W0 // 2                    # 84 even columns carrying x
U8 = mybir.dt.uint8
F16 = mybir.dt.float16
F32 = mybir.dt.float32

DEFAULT_VARIANT = dict(sparse_l0=True, half_l7=True)


def _build_program(reps=1, sparse_l0=True, half_l7=True, probe=""):
    nc = bacc.Bacc("TRN2", target_bir_lowering=False, debug=False)
    xs = nc.dram_tensor("xs", [128, XW], F16, kind="ExternalInput").ap()
    # uint8 affine-quantized toggles, one contiguous (combo, col) block per
    # (layer, chunk): [layer, chunk, combo*W0]
    tg = nc.dram_tensor("tg", [L, NCHUNK, P * W0], U8, kind="ExternalInput").ap()
    # layer-0 compact toggles: [chunk, parity*combo(8)*e]
    tg0 = nc.dram_tensor("tg0", [NCHUNK, 2 * 8 * XW], U8, kind="ExternalInput").ap()
    # layer-7 toggles for even output columns only: [chunk, combo*e]
    tg7 = nc.dram_tensor("tg7", [NCHUNK, P * (CHUNK // 2)], U8,
                         kind="ExternalInput").ap()
    # dequant [scale, bias] per partition (fp32), input-data dependent
    qsb = nc.dram_tensor("qsb", [128, 2], F32, kind="ExternalInput").ap()
    out = nc.dram_tensor("out", [128, CHUNK // 2], F32, kind="ExternalOutput").ap()

    mult = mybir.AluOpType.mult
    add = mybir.AluOpType.add
    AF = mybir.ActivationFunctionType

    with tile.TileContext(nc) as tc, ExitStack() as ctx:
        pool = ctx.enter_context(tc.tile_pool(name="work", bufs=1))
        tqpool = ctx.enter_context(tc.tile_pool(name="twq", bufs=3))
        tfpool = ctx.enter_context(tc.tile_pool(name="twf", bufs=2))

        # paired state tiles: row0 = comp (1-state), row1 = state
        SC = [pool.tile([128, 2, W0], F16, name="scA", tag="scA"),
              pool.tile([128, 2, W0], F16, name="scB", tag="scB")]
        t4 = pool.tile([128, 2, 2, W0], F16, name="t4", tag="t4")
        t23 = pool.tile([128, 2, 2, W0], F16, name="t23", tag="t23")
        t45 = pool.tile([128, 2, 2, W0], F16, name="t45", tag="t45")
        wb16 = pool.tile([128, 4, 4, W0], F16, name="wb16", tag="wb16")
        p64 = pool.tile([128, 2, 16, W0], F16, name="p64", tag="p64")
        gf = pool.tile([128, 4, W0], F16, name="gf", tag="gf")
        fin = pool.tile([128, 4, W0], F16, name="fin", tag="fin")
        # compact stride-1 parity copies of state for layer 0 / half layer 7
        cpar = pool.tile([128, 2, 2, XW], F16, name="cpar", tag="cpar")
        xt = pool.tile([128, XW], F16, name="xt", tag="xt")
        sbq = pool.tile([128, 2], F32, name="sbq", tag="sbq")
        o32 = pool.tile([128, CHUNK // 2], F32, name="o32", tag="o32")

        nc.gpsimd.dma_start(out=sbq[:], in_=qsb)
        qs, qb = sbq[:, 0:1], sbq[:, 1:2]

        if sparse_l0:
            nc.gpsimd.dma_start(out=xt[:], in_=xs[:, :])
        else:
            nc.vector.memset(SC[0][:], 0.0)
            nc.gpsimd.dma_start(out=SC[0][:, 1, 0:W0:2], in_=xs[:, :])

        twq_tiles = {}
        twf_tiles = {}

        def pruned(gl):
            return half_l7 and gl % L == L - 1

        def fetch_tw(gl):
            t = tqpool.tile([128, P * W0], U8, name="twt", tag="twq")
            if probe != "nodma" or gl <= 1:
                if pruned(gl):
                    hw = 32 * (CHUNK // 2)
                    for h in range(2):
                        nc.sync.dma_start(
                            out=t[:, h * hw:(h + 1) * hw],
                            in_=tg7[:, h * hw:(h + 1) * hw]
                            .partition_broadcast(16))
                else:
                    hw = 32 * W0
                    for h in range(2):
                        nc.sync.dma_start(
                            out=t[:, h * hw:(h + 1) * hw],
                            in_=tg[gl % L][:, h * hw:(h + 1) * hw]
                            .partition_broadcast(16))
            twq_tiles[gl] = t

        def sigmoid_tw(gl, half):
            if gl not in twf_tiles:
                twf_tiles[gl] = tfpool.tile([128, P, W0], F16, name="twf",
                                            tag="twf")
            if probe == "nosig":
                return
            tq, tf = twq_tiles[gl], twf_tiles[gl]
            if pruned(gl):
                w, lo, ro = CHUNK // 2, 0, CHUNK // 2
            else:
                ll = gl % L
                w, lo, ro = W0, 2 * ll + 2, W0 - 3 * ll - 3
            qv = tq.rearrange("p (q w) -> p q w", w=w)
            nc.scalar.activation(tf[:, 32 * half:32 * half + 32, lo:ro],
                                 qv[:, 32 * half:32 * half + 32, lo:ro],
                                 AF.Sigmoid, scale=qs, bias=qb)

        def needs_tw(gl):
            return gl < L * reps and not (sparse_l0 and gl % L == 0)

        if sparse_l0:
            tw0q = pool.tile([128, 2 * 8 * XW], U8, name="tw0q", tag="tw0q")
            tw0 = pool.tile([128, 2, 8, XW], F16, name="tw0", tag="tw0")
            nc.gpsimd.dma_start(out=tw0q[:], in_=tg0.partition_broadcast(16))
            nc.scalar.activation(tw0.rearrange("p a q e -> p (a q e)"),
                                 tw0q[:], AF.Sigmoid, scale=qs, bias=qb)
        else:
            fetch_tw(0)
            sigmoid_tw(0, 0)
            sigmoid_tw(0, 1)
        if needs_tw(1):
            fetch_tw(1)

        for gl in range(L * reps):
            l = gl % L
            lin, rin = 2 * l, W0 - 3 * l
            lo, ro = lin + 2, rin - 3
            wos = ro - lo
            sin, sout = SC[gl % 2], SC[(gl + 1) % 2]

            # prefetch toggle gates TWO layers ahead (bufs=3) so next layer's
            # sigmoid never waits on its DMA
            if needs_tw(gl + 2):
                fetch_tw(gl + 2)

            if not (sparse_l0 and l == 0):
                # comp = 1 - state on the input window (fp16 tensor_scalar: 4x)
                nc.vector.tensor_scalar(sin[:, 0, lin:rin], sin[:, 1, lin:rin],
                                        -1.0, 1.0, mult, add)

            # sigmoid queues on ACT in two 32-combo halves so consumer
            # big-muls gate on half the DMA + half the sigmoid
            if needs_tw(gl + 1):
                sigmoid_tw(gl + 1, 0)
                sigmoid_tw(gl + 1, 1)

            if sparse_l0 and l == 0:
                # Layer 0: odd grid slots are exactly 0 (state) / 1 (comp), so
                # only 8 of 64 combos survive per output parity; taps collapse
                # to stride-1 views of a COMPACT x tile cpar[:, 0] with
                # dim 0=comp, 1=state of the 84 x-carrying even slots.
                nc.vector.tensor_scalar(cpar[:, 0, 1, :], xt[:, :],
                                        1.0, 0.0, mult, add)
                nc.vector.tensor_scalar(cpar[:, 0, 0, :], cpar[:, 0, 1, :],
                                        -1.0, 1.0, mult, add)
                X = cpar[:, 0]  # [128, 2, XW]: dim1 0=comp, 1=state

                for par, ne in ((0, 82), (1, 81)):
                    # even outputs j=2e, e in [1,82]: taps X[e-1], X[e], X[e+1]
                    # odd outputs j=2e+1, e in [1,81]: taps X[e], X[e+1], X[e+2]
                    V = [X[:, :, d + par: d + par + ne] for d in (0, 1, 2)]
                    tp = t4[:, :, :, 0:ne]
                    nc.vector.tensor_tensor(
                        tp,
                        V[0].unsqueeze(2).broadcast_to((128, 2, 2, ne)),
                        V[1].unsqueeze(1).broadcast_to((128, 2, 2, ne)), mult)
                    w8 = wb16.rearrange("p a b j -> p (a b) j") \
                        .rearrange("p (q c) j -> p q c j", c=2)[:, 0:4, :, 0:ne]
                    nc.vector.tensor_tensor(
                        w8,
                        t4.rearrange("p a b j -> p (a b) j")[:, :, 0:ne]
                        .unsqueeze(2).broadcast_to((128, 4, 2, ne)),
                        V[2].unsqueeze(1).broadcast_to((128, 4, 2, ne)), mult)
                    tw0v = tw0[:, par].rearrange("p (q c) j -> p q c j", c=2)
                    nc.vector.tensor_tensor(w8, w8,
                                            tw0v[:, :, :, 1:1 + ne], mult)
                    nc.vector.tensor_tensor(w8[:, 0:2], w8[:, 0:2],
                                            w8[:, 2:4], add)
                    nc.vector.tensor_tensor(w8[:, 0, :, :], w8[:, 0, :, :],
                                            w8[:, 1, :, :], add)
                    nc.vector.tensor_tensor(
                        sout[:, 1, 2 + par:2 + par + 2 * ne:2],
                        w8[:, 0, 0, :], w8[:, 0, 1, :], add)
                continue

            twl = twf_tiles[gl]
            half7 = half_l7 and l == L - 1

            if half7:
                # compact stride-1 parity copies: even-col taps 0,2,4 and
                # odd-col taps 1,3,5 (output cols j=lo..ro step 2, wos evens)
                wos = wos // 2
                nce = wos + 3
                nc.vector.tensor_scalar(
                    cpar[:, 0, :, 0:nce],
                    sin[:, :, lin: lin + 2 * nce: 2], 1.0, 0.0, mult, add)
                nc.vector.tensor_scalar(
                    cpar[:, 1, :, 0:nce],
                    sin[:, :, lin + 1: lin + 1 + 2 * nce: 2], 1.0, 0.0,
                    mult, add)

                def VP(i):
                    return cpar[:, i % 2, :, i // 2: i // 2 + wos]
            else:
                def VP(i):
                    return sin[:, :, lin + i: lin + i + wos]

            # --- 2+4 bit split: wA = taps 0,1 (4 combos, = t4),
            #     wB16 = taps 2..5 (16 combos) from two pair trees, combo-major
            nc.vector.tensor_tensor(
                t4[:, :, :, 0:wos],
                VP(0).unsqueeze(2).broadcast_to((128, 2, 2, wos)),
                VP(1).unsqueeze(1).broadcast_to((128, 2, 2, wos)), mult)
            nc.vector.tensor_tensor(
                t23[:, :, :, 0:wos],
                VP(2).unsqueeze(2).broadcast_to((128, 2, 2, wos)),
                VP(3).unsqueeze(1).broadcast_to((128, 2, 2, wos)), mult)
            nc.vector.tensor_tensor(
                t45[:, :, :, 0:wos],
                VP(4).unsqueeze(2).broadcast_to((128, 2, 2, wos)),
                VP(5).unsqueeze(1).broadcast_to((128, 2, 2, wos)), mult)
            nc.vector.tensor_tensor(
                wb16[:, :, :, 0:wos],
                t23.rearrange("p a b j -> p (a b) j")[:, :, 0:wos]
                .unsqueeze(2).broadcast_to((128, 4, 4, wos)),
                t45.rearrange("p a b j -> p (a b) j")[:, :, 0:wos]
                .unsqueeze(1).broadcast_to((128, 4, 4, wos)), mult)

            # --- per tw-half: products then pairwise pb add-tree (all views
            #     keep j innermost stride-1 -> every op runs the fp16 2x path)
            for h in range(2):
                tv = (twl[:, 32 * h:32 * h + 32, 0:wos] if half7 else
                      twl[:, 32 * h:32 * h + 32, lo:ro])
                nc.vector.tensor_tensor(
                    p64[:, :, :, 0:wos],
                    wb16.rearrange("p a b j -> p (a b) j")[:, :, 0:wos]
                    .unsqueeze(1).broadcast_to((128, 2, 16, wos)),
                    tv.rearrange("p (a b) j -> p a b j", a=2), mult)
                for w_ in (8, 4, 2):
                    nc.vector.tensor_tensor(
                        p64[:, :, 0:w_, 0:wos], p64[:, :, 0:w_, 0:wos],
                        p64[:, :, w_:2 * w_, 0:wos], add)
                nc.vector.tensor_tensor(
                    gf[:, 2 * h:2 * h + 2, 0:wos],
                    p64[:, :, 0, 0:wos], p64[:, :, 1, 0:wos], add)

            # --- out = sum_{pa in 4} wA[pa] * g[pa] ---
            nc.vector.tensor_tensor(
                fin[:, :, 0:wos],
                t4.rearrange("p a b j -> p (a b) j")[:, :, 0:wos],
                gf[:, :, 0:wos], mult)
            nc.vector.tensor_tensor(fin[:, 0:2, 0:wos], fin[:, 0:2, 0:wos],
                                    fin[:, 2:4, 0:wos], add)
            if half7:
                # layer 7 computes exactly the owned even columns: write the
                # fp32 output tile directly
                nc.vector.tensor_tensor(o32[:, :], fin[:, 0, 0:wos],
                                        fin[:, 1, 0:wos], add)
            else:
                nc.vector.tensor_tensor(sout[:, 1, lo:ro], fin[:, 0, 0:wos],
                                        fin[:, 1, 0:wos], add)

        if not half_l7:
            # owned even columns -> fp32 output
            nc.vector.tensor_scalar(
                o32[:, :], SC[(L * reps) % 2][:, 1, GROW_L:GROW_L + CHUNK:2],
                1.0, 0.0, mult, add)
        nc.sync.dma_start(out=out, in_=o32[:, :])

    nc.compile()
    return nc


_prog_cache = {}


def _get_program(reps=1, **variant):
    v = dict(DEFAULT_VARIANT)
    v.update(variant)
    key = (reps, tuple(sorted(v.items())))
    if key not in _prog_cache:
        _prog_cache[key] = _build_program(reps, **v)
    return _prog_cache[key]


def _shard_inputs(x, toggle_gates):
    x = np.ascontiguousarray(x, dtype=np.float32)
    tg = np.ascontiguousarray(toggle_gates, dtype=np.float32)
    # affine uint8 quantization of the raw gates (exactly invertible at the
    # device dequant: g ~ lo + q*(hi-lo)/255, shipped as per-partition scale/
    # bias so the compiled program stays input-independent)
    lo, hi = float(tg.min()), float(tg.max())
    scale = (hi - lo) / 255.0 if hi > lo else 1.0
    tgq8 = np.round((tg - lo) / scale).astype(np.uint8)
    qsb = np.tile(np.array([[scale, lo]], np.float32), (128, 1))
    in_maps = []
    c = np.arange(NCHUNK)
    j = np.arange(W0)
    # layer-0 surviving combos (even outputs: bits 1,3,5 = 0; odd: bits 0,2,4 = 0)
    p_even = np.array([32 * (q >> 2) + 8 * ((q >> 1) & 1) + 2 * (q & 1)
                       for q in range(8)])
    p_odd = np.array([16 * (q >> 2) + 4 * ((q >> 1) & 1) + (q & 1)
                      for q in range(8)])
    for k in range(NCORES):
        n0 = k * NOWN
        nglob = (n0 + CHUNK * c[:, None] - GROW_L + j[None, :]) % N  # [8, 168]
        m_idx = nglob[:, 0::2] // 2                                   # [8, 84]
        xs = x[:, m_idx].reshape(B * NCHUNK, XW)                      # [128, 84]
        tgk = tgq8[:, :, nglob]                                       # [L, P, 8, 168]
        tg0 = np.stack([tgk[0, p_even][:, :, 0::2],                   # [8q, 8c, 84]
                        tgk[0, p_odd][:, :, 1::2]])                   # [2, 8q, 8c, 84]
        tg0 = np.ascontiguousarray(tg0.transpose(2, 0, 1, 3))         # [8c, 2, 8q, 84]
        tg7 = np.ascontiguousarray(
            tgk[L - 1][:, :, GROW_L:GROW_L + CHUNK:2].transpose(1, 0, 2))  # [8c,P,64]
        tgk = np.ascontiguousarray(tgk.transpose(0, 2, 1, 3))         # [L, 8, P, 168]
        in_maps.append({"xs": np.ascontiguousarray(xs).astype(np.float16),
                        "tg": tgk.reshape(L, NCHUNK, P * W0),
                        "tg0": tg0.reshape(NCHUNK, 2 * 8 * XW),
                        "tg7": tg7.reshape(NCHUNK, P * (CHUNK // 2)),
                        "qsb": qsb})
    return in_maps


def _run(x, toggle_gates, trace=False, reps=1, **kw):
    nc = _get_program(reps, **kw)
    in_maps = _shard_inputs(x, toggle_gates)
    res = run_bass_kernel_spmd(nc, in_maps, list(range(NCORES)), trace=trace)
    y = np.empty((B, M), dtype=np.float32)
    for k in range(NCORES):
        o = np.asarray(res.results[k]["out"]).reshape(B, NCHUNK * CHUNK // 2)
        y[:, k * (NOWN // 2):(k + 1) * (NOWN // 2)] = o
    return y, res


def kernel(x, toggle_gates):
    # Retry-then-fallback ladder: a transient device error (e.g.
    # NRT_EXEC_UNIT_UNRECOVERABLE was observed during development) should
    # not zero the run.  The fastest variant is tried twice before stepping
    # down to the plainer one.
    ladder = [
        dict(DEFAULT_VARIANT),
        dict(DEFAULT_VARIANT, sparse_l0=False, half_l7=False),
    ]
    last_err = None
    for v in ladder:
        for _attempt in range(2):
            try:
                y, _ = _run(x, toggle_gates, **v)
                return y
            except Exception as e:  # noqa: BLE001 - deliberate catch-all retry
                last_err = e
    raise last_err
